# revision 1
# baseline (speedup 1.0000x reference)
"""Tucker-style 3-mode contraction kernel for Trainium2 (8 NeuronCores).

Problem: x [1024*32*32*32] fp32, w0/w1/w2 [32,32] fp32.
  out[B,A,Bb,C] = sum_{a,b,c} x[B,a,b,c] w0[a,A] w1[b,Bb] w2[c,C]

Strategy (per core, data-parallel over batch, 128 batch elems/core):
  - Sub-tile: 4 batch elems ("groups" g) x full 32x32x32 tensor each
    -> SBUF tile [128 partitions = (g, mode), 1024 free].
  - Stationary weights are diag4 = kron(I4, w) [128,128]: one matmul
    contracts the partition-mode of all 4 groups at once (2x N=512).
  - Between stages, DVE StreamTranspose on dense 2D tiles (32x32 blocks)
    moves the inner free mode onto partitions; strided matmul rhs views
    pre-swap the free order so the right mode is inner:
      MM1 (w0): X [(g,a),(b,c)]        -> psum1 [(g,A),(b,c)]
      T1:                               -> sbuf z1t [(g,c),(b,A)]
      MM2 (w2, rhs viewed (A,b)):       -> psum2 [(g,C),(A,b)]
      T2:                               -> sbuf z2t [(g,b),(A,C)]
      MM3 (w1, rhs viewed (C,A)):       -> psum3 [(g,B),(C,A)]
      T3:                               -> sbuf z3t [(g,A),(C,B)]
      ACT copy (in viewed (B,C)):       -> Y [(g,A),(B,C)]   (dense DMA out)
  - Super-tiles of 4 sub-tiles (16 batch elems) give 2 MiB contiguous DMAs.
"""

import os

import numpy as np

N_CORES = 8
BATCH = 1024
F = 32  # factor dim
ELEM = F * F * F  # 32768 elems per batch element
B_PER_CORE = BATCH // N_CORES  # 128
G = 4  # batch groups per sub-tile (4*32 = 128 partitions)
S = 4  # sub-tiles per super-tile
T = B_PER_CORE // (G * S)  # 8 super-tiles per core
FF = F * F  # 1024

# "float32" (exact, PE 4 cyc/row) or "float32r" (TF32-like, 1 cyc/row)
MM_DTYPE = os.environ.get("KERNEL_MM_DTYPE", "float32")

_CACHE = {}


def build_program(mm_dtype=MM_DTYPE, repeat=1):
    key = (mm_dtype, repeat)
    if key in _CACHE:
        return _CACHE[key]

    import concourse.bacc as bacc
    import concourse.mybir as mybir
    import concourse.tile as tile

    f32 = mybir.dt.float32
    mmdt = getattr(mybir.dt, mm_dtype)

    nc = bacc.Bacc("TRN2", target_bir_lowering=False, debug=False,
                   num_devices=N_CORES)

    xs = nc.dram_tensor("xs", [T, S, G, F, FF], mmdt, kind="ExternalInput")
    w0d = nc.dram_tensor("w0d", [128, 128], mmdt, kind="ExternalInput")
    w1d = nc.dram_tensor("w1d", [128, 128], mmdt, kind="ExternalInput")
    w2d = nc.dram_tensor("w2d", [128, 128], mmdt, kind="ExternalInput")
    ys = nc.dram_tensor("ys", [T, S, G, F, FF], f32, kind="ExternalOutput")

    def mm(out_ap, lhsT_ap, rhs_ap):
        nc.tensor.matmul(out_ap, lhsT_ap, rhs_ap, start=True, stop=True)

    with tile.TileContext(nc) as tc:
        with (
            tc.tile_pool(name="consts", bufs=1) as cpool,
            tc.tile_pool(name="xp", bufs=2) as xp,
            tc.tile_pool(name="yp", bufs=2) as yp,
            tc.tile_pool(name="zt", bufs=3) as ztp,
            tc.tile_pool(name="ps1", bufs=2, space="PSUM") as ps1,
            tc.tile_pool(name="ps2", bufs=1, space="PSUM") as ps2,
            tc.tile_pool(name="ps3", bufs=1, space="PSUM") as ps3,
        ):
            w0t = cpool.tile([128, 128], mmdt)
            w1t = cpool.tile([128, 128], mmdt)
            w2t = cpool.tile([128, 128], mmdt)
            nc.sync.dma_start(out=w0t[:], in_=w0d[:])
            nc.sync.dma_start(out=w1t[:], in_=w1d[:])
            nc.sync.dma_start(out=w2t[:], in_=w2d[:])

            for t in range(T * repeat):
                t = t % T
                X = xp.tile([128, S, FF], mmdt)  # [(g,a), s, (b,c)]
                nc.sync.dma_start(
                    out=X[:], in_=xs[t].rearrange("s g a m -> (g a) s m"))
                Y = yp.tile([128, S, F, F], f32)  # [(g,A), s, B, C]
                for s in range(S):
                    # stage 1: contract a -> psum1 [(g,A), (b,c)]
                    z1 = ps1.tile([128, FF], mmdt, tag="z1")
                    mm(z1[:, 0:512], w0t[:], X[:, s, 0:512])
                    mm(z1[:, 512:1024], w0t[:], X[:, s, 512:1024])
                    # T1: -> [(g,c), (b,A)]
                    z1t = ztp.tile([128, FF], mmdt, tag="z1t")
                    nc.vector.transpose(out=z1t[:], in_=z1[:])
                    # stage 2: contract c; rhs viewed (A,b) -> psum2 [(g,C),(A,b)]
                    z1v = z1t[:].rearrange("p (b a) -> p a b", b=F, a=F)
                    z2 = ps2.tile([128, FF], mmdt, tag="z2")
                    mm(z2[:, 0:512], w2t[:], z1v[:, 0:F // 2, :])
                    mm(z2[:, 512:1024], w2t[:], z1v[:, F // 2:F, :])
                    # T2: -> [(g,b), (A,C)]
                    z2t = ztp.tile([128, FF], mmdt, tag="z2t")
                    nc.vector.transpose(out=z2t[:], in_=z2[:])
                    # stage 3: contract b; rhs viewed (C,A) -> psum3 [(g,B),(C,A)]
                    z2v = z2t[:].rearrange("p (a c) -> p c a", a=F, c=F)
                    z3 = ps3.tile([128, FF], f32, tag="z3")
                    mm(z3[:, 0:512], w1t[:], z2v[:, 0:F // 2, :])
                    mm(z3[:, 512:1024], w1t[:], z2v[:, F // 2:F, :])
                    # T3: -> z3t [(g,A), (C,B)]
                    z3t = ztp.tile([128, FF], f32, tag="z3t")
                    nc.vector.transpose(out=z3t[:], in_=z3[:])
                    # final free reorder (C,B) -> (B,C) on ScalarE
                    nc.scalar.copy(
                        out=Y[:, s],
                        in_=z3t[:].rearrange("p (c b) -> p b c", c=F, b=F))
                nc.scalar.dma_start(
                    out=ys[t].rearrange("s g a (b c) -> (g a) s b c", b=F, c=F),
                    in_=Y[:])

    nc.compile()
    _CACHE[key] = nc
    return nc


def _diag4(w):
    return np.kron(np.eye(G, dtype=np.float32), np.asarray(w, np.float32))


def make_in_maps(x, w0, w1, w2):
    x = np.ascontiguousarray(np.asarray(x, np.float32).reshape(-1))
    assert x.size == BATCH * ELEM
    shards = x.reshape(N_CORES, T, S, G, F, FF)
    w0d, w1d, w2d = _diag4(w0), _diag4(w1), _diag4(w2)
    return [
        {"xs": shards[i], "w0d": w0d, "w1d": w1d, "w2d": w2d}
        for i in range(N_CORES)
    ]


def kernel(x, w0, w1, w2, trace=False):
    from concourse.bass_utils import run_bass_kernel_spmd

    nc = build_program()
    in_maps = make_in_maps(x, w0, w1, w2)
    res = run_bass_kernel_spmd(nc, in_maps, core_ids=list(range(N_CORES)),
                               trace=trace)
    out = np.concatenate([res.results[i]["ys"].reshape(-1)
                          for i in range(N_CORES)])
    if trace:
        return out, res
    return out



# revision 10
# speedup vs baseline: 1.8495x; 1.8495x over previous
"""Tucker-style 3-mode contraction kernel for Trainium2 (8 NeuronCores).

Problem: x [1024*32*32*32] fp32, w0/w1/w2 [32,32] fp32.
  out[B,A,Bb,C] = sum_{a,b,c} x[B,a,b,c] w0[a,A] w1[b,Bb] w2[c,C]

Design (v3, fp16 end-to-end; data-parallel over batch, 128 elems/core):
  - Host pre-permutes x so each sub-tile (4 batch elems g) lands in SBUF as
    [(g,c) partitions, (a,b) free] fp16 -- contraction order c, b, a.
  - Each matmul contracts the partition dim with a host-baked 128x128
    "stationary" that fuses the 32x32 factor with an arbitrary slot
    permutation (any 128x128 matrix is legal):
      MM1: S1 = kron(I4, w2): (g,c)->(g,C);     psum1 [(g,C),(a,b)]
      MM2: S2 = kron(I4, w1): (a2,b)->(a2,B);   psum2 [(a2,B),(C,g,ah)]
      MM3: S3 = w0 perm:  (a2,g,ah)->(g,A);     psum3 [(g,A),(B,C)]
    (a = 4*ah + a2; a2 on partitions after the PE transpose.)
  - Cross-partition moves split across engines:
      move1: ACT copy psum1->s1 fp16; PE transpose (8x 128x128 chunks)
             s1 -> psumT fp16; DVE copy psumT -> z1 (2-byte 2x mode)
      move2: DVE StreamTranspose psum2 (fp32 -> fp32, ISA requires same
             dtype); GPSIMD casts fp32 -> fp16 for MM3 (GPSIMD is idle
             otherwise and cannot touch PSUM, but SBUF->SBUF is fine)
  - MM3's output layout is already [(g,A),(B,C)]: ACT copies psum3 -> Y fp16
    and DMA-out is dense (8KB/partition lines). Host un-permutes + upcasts.
  - I/O is fp16 (halves HBM traffic); PSUM accumulation stays fp32.
"""

import numpy as np

N_CORES = 8
BATCH = 1024
F = 32
FF = F * F  # 1024
T = 8   # super-tiles per core
S = 4   # sub-tiles per super-tile
G = 4   # batch elems per sub-tile (128 = T*S*G per core)

_CACHE = {}


def build_program():
    if "nc" in _CACHE:
        return _CACHE["nc"]

    import concourse.bacc as bacc
    import concourse.mybir as mybir
    import concourse.tile as tile

    f32 = mybir.dt.float32
    f16 = mybir.dt.float16
    f32r = mybir.dt.float32r

    nc = bacc.Bacc("TRN2", target_bir_lowering=False, debug=False,
                   num_devices=N_CORES)

    xs = nc.dram_tensor("xs", [T, 128, S, FF], f16, kind="ExternalInput")
    s1d = nc.dram_tensor("s1d", [128, 128], f16, kind="ExternalInput")
    s2d = nc.dram_tensor("s2d", [128, 128], f16, kind="ExternalInput")
    s3d = nc.dram_tensor("s3d", [128, 128], f16, kind="ExternalInput")
    idd = nc.dram_tensor("idd", [128, 128], f16, kind="ExternalInput")
    ys = nc.dram_tensor("ys", [T, 128, S, FF], f16, kind="ExternalOutput")

    def mm(out_ap, lhsT_ap, rhs_ap):
        nc.tensor.matmul(out_ap, lhsT_ap, rhs_ap, start=True, stop=True)

    with tile.TileContext(nc) as tc:
        with (
            tc.tile_pool(name="consts", bufs=1) as cpool,
            tc.tile_pool(name="xp", bufs=2) as xp,
            tc.tile_pool(name="yp", bufs=2) as yp,
            tc.tile_pool(name="s1p", bufs=2) as s1p,
            tc.tile_pool(name="z1p", bufs=2) as z1p,
            tc.tile_pool(name="z2p", bufs=2) as z2p,
            tc.tile_pool(name="z2hp", bufs=2) as z2hp,
            tc.tile_pool(name="ps1", bufs=1, space="PSUM") as ps1,
            tc.tile_pool(name="psT", bufs=1, space="PSUM") as psT,
            tc.tile_pool(name="ps2", bufs=1, space="PSUM") as ps2,
            tc.tile_pool(name="ps3", bufs=1, space="PSUM") as ps3,
        ):
            S1t = cpool.tile([128, 128], f16)
            S2t = cpool.tile([128, 128], f16)
            S3t = cpool.tile([128, 128], f16)
            It = cpool.tile([128, 128], f16)
            nc.sync.dma_start(out=S1t[:], in_=s1d[:])
            nc.sync.dma_start(out=S2t[:], in_=s2d[:])
            nc.sync.dma_start(out=S3t[:], in_=s3d[:])
            nc.sync.dma_start(out=It[:], in_=idd[:])

            for t in range(T):
                X = xp.tile([128, S, FF], f16)  # [(g,c), s, (a,b)]
                nc.sync.dma_start(out=X[:], in_=xs[t])
                Y = yp.tile([128, S, FF], f16)  # [(g,A), s, (B,C)]
                for s in range(S):
                    # MM1: contract c -> psum1 [(g,C), (a,b)]
                    p1 = ps1.tile([128, FF], f32, tag="p1")
                    mm(p1[:, 0:512], S1t[:], X[:, s, 0:512])
                    mm(p1[:, 512:1024], S1t[:], X[:, s, 512:1024])
                    # ACT: fp32 psum -> fp16 sbuf
                    s1 = s1p.tile([128, FF], f16, tag="s1")
                    nc.scalar.copy(out=s1[:], in_=p1[:])
                    # PE transpose, 8 chunks of 128: -> [(a2,b), (ah,g,C)]
                    pT = psT.tile([128, 8, 128], f16, tag="pT")
                    for j in range(8):
                        nc.tensor.transpose(
                            pT[:, j], s1[:, 128 * j:128 * (j + 1)], It[:])
                    # DVE: psum fp16 -> sbuf fp16 (2x mode)
                    z1 = z1p.tile([128, 8, 128], f16, tag="z1")
                    nc.vector.tensor_copy(out=z1[:], in_=pT[:])
                    # MM2: contract b; rhs streamed (C, g, ah)
                    #   -> psum2 [(a2,B), (C,(g,ah))]
                    z1v = z1[:].rearrange("p ah (g c) -> p c g ah", g=G, c=F)
                    p2 = ps2.tile([128, FF], f32, tag="p2")
                    mm(p2[:, 0:512], S2t[:], z1v[:, 0:16])
                    mm(p2[:, 512:1024], S2t[:], z1v[:, 16:32])
                    # DVE StreamTranspose (32x32 blocks), fp32 -> fp32
                    # (ISA: src dtype must equal dst dtype):
                    #   -> z2w [(a2,(g,ah)), (C,B)]
                    z2w = z2p.tile([128, FF], f32, tag="z2w")
                    nc.vector.transpose(out=z2w[:], in_=p2[:])
                    # GPSIMD (otherwise idle): fp32 -> fp16 cast, SBUF->SBUF
                    z2 = z2hp.tile([128, FF], f16, tag="z2")
                    nc.gpsimd.tensor_copy(out=z2[:], in_=z2w[:])
                    # MM3: contract a; rhs streamed (B, C)
                    #   -> psum3 [(g,A), (B,C)]
                    z2v = z2[:].rearrange("p (c b) -> p b c", c=F, b=F)
                    p3 = ps3.tile([128, FF], f32, tag="p3")
                    mm(p3[:, 0:512], S3t[:], z2v[:, 0:16])
                    mm(p3[:, 512:1024], S3t[:], z2v[:, 16:32])
                    # ACT: fp32 psum -> fp16 Y
                    nc.scalar.copy(out=Y[:, s], in_=p3[:])
                nc.sync.dma_start(out=ys[t], in_=Y[:])

    nc.compile()
    _CACHE["nc"] = nc
    return nc


def make_in_maps(x, w0, w1, w2):
    x = np.asarray(x, np.float32).reshape(N_CORES, T, S, G, F, F, F)
    # [core, t, s, g, a, b, c] -> [core, t, g, c, s, a, b]
    xs_all = x.transpose(0, 1, 3, 6, 2, 4, 5).astype(np.float16)
    xs_all = np.ascontiguousarray(xs_all).reshape(N_CORES, T, 128, S, FF)

    w0 = np.asarray(w0, np.float32)
    w1 = np.asarray(w1, np.float32)
    w2 = np.asarray(w2, np.float32)
    eye4 = np.eye(4, dtype=np.float32)
    S1 = np.kron(eye4, w2).astype(np.float16)  # (g,c) -> (g,C)
    S2 = np.kron(eye4, w1).astype(np.float16)  # (a2,b) -> (a2,B)
    S3 = np.zeros((128, 128), np.float32)      # (a2,g,ah) -> (g,A)
    for a2 in range(4):
        for g in range(4):
            for ah in range(8):
                S3[a2 * 32 + g * 8 + ah, g * 32:(g + 1) * 32] = \
                    w0[ah * 4 + a2, :]
    S3 = S3.astype(np.float16)
    I128 = np.eye(128, dtype=np.float16)

    return [
        {"xs": xs_all[i], "s1d": S1, "s2d": S2, "s3d": S3, "idd": I128}
        for i in range(N_CORES)
    ]


def kernel(x, w0, w1, w2, trace=False):
    from concourse.bass_utils import run_bass_kernel_spmd

    nc = build_program()
    in_maps = make_in_maps(x, w0, w1, w2)
    res = run_bass_kernel_spmd(nc, in_maps, core_ids=list(range(N_CORES)),
                               trace=trace)
    outs = []
    for i in range(N_CORES):
        y = res.results[i]["ys"].reshape(T, G, F, S, F, F)  # [t,g,A,s,B,C]
        y = y.transpose(0, 3, 1, 2, 4, 5)                   # [t,s,g,A,B,C]
        outs.append(np.ascontiguousarray(y).reshape(-1).astype(np.float32))
    out = np.concatenate(outs)
    if trace:
        return out, res
    return out
